# revision 23
# baseline (speedup 1.0000x reference)
"""PreconditionerSparseUNet on 8 TRN2 NeuronCores.

Sharding: data-parallel over batch (8 images, 1 per core). Each core runs the
full U-Net on its own 512x512x1 image; weights are replicated.

Per-core implementation notes (v3 — HBM-traffic-minimized):
- Feature maps live in DRAM UNPADDED as [C, w*w] fp16. No halo ring is ever
  stored: loads materialize a padded (w+2)-pitch window in SBUF, where ring
  columns / halo rows are zeroed by tiny strided memsets per group.
- Each conv group loads its input rows from HBM ONCE; the shifted partition
  blocks needed for K-folding are produced by on-chip SBUF->SBUF copies
  (kills the 2-3x HBM read amplification of the naive scheme).
- 3x3 convs are a few matmuls per output chunk: input channel blocks
  replicated into SBUF partitions at shifted offsets so one matmul contracts
  several taps. rhs/lhsT fp16 (full PE rate), PSUM accumulates fp32.
- Conv outputs of M<=64 channels are packed 4x/2x along PSUM partitions so
  the bias+LeakyReLU epilogue runs once per 128-partition tile. Output rows
  are row-aligned: stores are contiguous ~1KB runs into the unpadded layout.
- Transposed convs are decomposed into 4 output-parity classes written with
  stride-2 row access; skip-add happens after LeakyReLU, before store.
- The final 1x1 conv + triangular masking is FUSED into dec0's epilogue
  (dec0's feature map never round-trips DRAM); the diagonal gets a softplus
  patch at the end.
"""

import os

import numpy as np

import concourse.bass as bass
import concourse.bacc as bacc
import concourse.mybir as mybir
from concourse.tile import TileContext
from concourse.bass_utils import run_bass_kernel_spmd

AF = mybir.ActivationFunctionType
F32 = mybir.dt.float32
F16 = mybir.dt.float16

STREAM_DT = F16
STREAM_NP = np.float16

N = 512
B = 8
ALPHA = 0.01

CH = [1, 16, 32, 64, 128, 1]


def wp(w):
    return w + 2


# ----------------------------------------------------------------------------
# Matmul plans. Each matmul: dict(p0, segs, r) where segs is a list of
# (ky, kx) weight-slice picks (or None for zero rows), one per Cin-sized
# block slot starting at partition p0; r is the rhs offset in padded-window
# elements relative to the chunk base.
# ----------------------------------------------------------------------------

# layers whose staging skips shifted-block replication: conv taps become 9
# accumulating K=cin matmuls with rhs offsets (cheap PE) instead of 2-3x
# SBUF->SBUF copy traffic (expensive DMA under activity throttling)
NOBLK = {"down1", "up0"}


def s1_plan(cin, w, noblk=False):
    W = wp(w)
    if noblk:
        blocks = [0]
        mms = [dict(p0=0, segs=[(ky, kx)], r=ky * W + kx)
               for ky in range(3) for kx in range(3)]
        return blocks, mms
    if cin == 1:
        blocks = [ky * W + kx for ky in range(3) for kx in range(3)]
        mms = [dict(p0=0, segs=[(ky, kx) for ky in range(3) for kx in range(3)], r=0)]
    elif cin <= 32:
        blocks = [0, 1, 2]
        mms = [dict(p0=0, segs=[(ky, 0), (ky, 1), (ky, 2)], r=ky * W)
               for ky in range(3)]
    elif cin == 64:
        blocks = [0, 1]
        mms = []
        for ky in range(3):
            mms.append(dict(p0=0, segs=[(ky, 0), (ky, 1)], r=ky * W))
            mms.append(dict(p0=0, segs=[(ky, 2)], r=ky * W + 2))
    else:
        raise ValueError(cin)
    return blocks, mms


def s2_plan(cin, w_in, noblk=False):
    # identical tap algebra to s1 (offsets are in padded input-window space)
    return s1_plan(cin, w_in, noblk)


def pmap(parity, d):
    # transposed conv: output parity p, input-tap delta d -> kernel index
    if parity == 0:
        return 1 if d == 0 else None
    return 0 if d == 0 else 2


def tconv_plan(cin, w_in, noblk=False):
    W = wp(w_in)
    if noblk or cin == 128:
        blocks = [0]
    elif cin == 64:
        blocks = [0, 1]  # dj shifts
    elif cin == 32:
        blocks = [0, 1, W, W + 1]  # (di,dj) = (0,0),(0,1),(1,0),(1,1)
    else:
        raise ValueError(cin)
    classes = []
    for py in range(2):
        for px in range(2):
            dis = [d for d in range(2) if pmap(py, d) is not None]
            djs = [d for d in range(2) if pmap(px, d) is not None]
            mms = []
            if noblk or cin == 128:
                for di in dis:
                    for dj in djs:
                        mms.append(dict(p0=0, segs=[(pmap(py, di), pmap(px, dj))],
                                        r=di * W + dj))
            elif cin == 64:
                for di in dis:
                    if len(djs) == 2:
                        mms.append(dict(p0=0,
                                        segs=[(pmap(py, di), pmap(px, 0)),
                                              (pmap(py, di), pmap(px, 1))],
                                        r=di * W))
                    else:
                        mms.append(dict(p0=0, segs=[(pmap(py, di), 1)],
                                        r=di * W))
            else:  # cin == 32, all four blocks resident
                if py == 0 and px == 0:
                    mms = [dict(p0=0, segs=[(1, 1)], r=0)]
                elif py == 0 and px == 1:
                    mms = [dict(p0=0, segs=[(1, 0), (1, 2)], r=0)]
                elif py == 1 and px == 0:
                    # one K=128 matmul over all four blocks; blocks holding
                    # dj=1 shifts get zero weight rows
                    mms = [dict(p0=0, segs=[(0, 1), None, (2, 1), None], r=0)]
                else:
                    mms = [dict(p0=0, segs=[(0, 0), (0, 2), (2, 0), (2, 2)], r=0)]
            classes.append((py, px, mms))
    return blocks, classes


# Layer table: (name, kind, cin, cout, w_in, w_out, wsrc, in, out, skip)
LAYERS = [
    ("enc1", "s1", 1, 16, 512, 512, "w_enc1", "x", "enc1p", None),
    ("down1", "s2", 16, 32, 512, 256, "w_down1", "enc1p", "down1p", None),
    ("enc2", "s1", 32, 32, 256, 256, "w_enc2", "down1p", "enc2p", None),
    ("down2", "s2", 32, 64, 256, 128, "w_down2", "enc2p", "down2p", None),
    ("enc3", "s1", 64, 64, 128, 128, "w_enc3", "down2p", "enc3p", None),
    ("bn", "s2", 64, 128, 128, 64, "w_bn", "enc3p", "bnp", None),
    ("up2", "tc", 128, 64, 64, 128, "w_up2", "bnp", "up2p", "enc3p"),
    ("dec2", "s1", 64, 64, 128, 128, "w_dec2", "up2p", "dec2p", None),
    ("up1", "tc", 64, 32, 128, 256, "w_up1", "dec2p", "up1p", "enc2p"),
    ("dec1", "s1", 32, 32, 256, 256, "w_dec1", "up1p", "dec1p", None),
    ("up0", "tc", 32, 16, 256, 512, "w_up0", "dec1p", "up0p", "enc1p"),
    ("dec0", "s1", 16, 16, 512, 512, "w_dec0", "up0p", None, None),
]

# group sizes: s1 = output rows; s2 = OUTPUT rows; tc = input rows
GROUP = {
    "enc1": 32, "down1": 16, "enc2": 32, "down2": 16, "enc3": 64,
    "bn": 16, "up2": 64, "dec2": 64, "up1": 32, "dec1": 32,
    "up0": 16, "dec0": 32,
}

# s1 rows per psum column-chunk (rpc * w <= 512)
S1_RPC = {512: 1, 256: 2, 128: 4}


def pack_stride(cout):
    return 32 if cout <= 32 else (64 if cout == 64 else 128)


def layer_plan(kind, cin, w_in, name=None):
    noblk = name in NOBLK
    if kind == "s1":
        return s1_plan(cin, w_in, noblk)
    if kind == "s2":
        return s2_plan(cin, w_in, noblk)
    return tconv_plan(cin, w_in, noblk)


def mm_keys(name, kind, cin, w_in):
    """Enumerate (key, m) for every matmul of a layer."""
    out = []
    if kind in ("s1", "s2"):
        _, mms = layer_plan(kind, cin, w_in, name)
        for i, m in enumerate(mms):
            out.append((f"W_{name}_{i}", m))
    else:
        _, classes = layer_plan(kind, cin, w_in, name)
        for py, px, mms in classes:
            for i, m in enumerate(mms):
                out.append((f"W_{name}_c{py}{px}_{i}", m))
    return out


# ----------------------------------------------------------------------------
# Host-side input prep
# ----------------------------------------------------------------------------

def prep_weights(inputs):
    """Build per-matmul lhsT arrays (fp16) and packed bias vectors."""
    wmap = {}
    for (name, kind, cin, cout, w_in, w_out, wsrc, *_rest) in LAYERS:
        w = np.asarray(inputs[wsrc])  # [3,3,cin,cout]
        for key, m in mm_keys(name, kind, cin, w_in):
            segs = []
            for s in m["segs"]:
                if s is None:
                    segs.append(np.zeros((cin, cout), np.float32))
                else:
                    segs.append(w[s[0], s[1]])
            wmap[key] = np.ascontiguousarray(
                np.concatenate(segs, axis=0).astype(STREAM_NP))
        bsrc = "b_" + wsrc[2:]
        b = np.asarray(inputs[bsrc]).astype(np.float32)
        stride = pack_stride(cout)
        b128 = np.zeros((128, 1), np.float32)
        for pk in range(128 // stride):
            b128[pk * stride: pk * stride + cout, 0] = b
        wmap[f"B_{name}"] = b128
    # final 1x1 conv: replicate the [16,1] weight at 4 partition bases so the
    # fused out-matmuls can run on diagonal PE tiles against packed act rows
    w_out4 = np.zeros((128, 1), np.float32)
    for pk in range(4):
        w_out4[pk * 32: pk * 32 + 16, 0] = np.asarray(inputs["w_out"]).reshape(16)
    wmap["W_out4"] = w_out4.astype(STREAM_NP)
    wmap["B_out"] = np.full((128, 1), float(np.asarray(inputs["b_out"])[0]),
                            np.float32)
    return wmap


# ----------------------------------------------------------------------------
# Kernel builder
# ----------------------------------------------------------------------------

def sub_ap(base_ap, p0, np_, off, dims):
    """AP over partitions [p0,p0+np_) of base_ap with free dims at elem off."""
    pitch = base_ap.ap[0][0]
    return bass.AP(base_ap.tensor, base_ap.offset + p0 * pitch + off,
                   [[pitch, np_]] + [list(d) for d in dims])


def strided_part_ap(base_ap, p0, pstep, np_, off, dims):
    """AP over partitions p0, p0+pstep, ... of base_ap."""
    pitch = base_ap.ap[0][0]
    return bass.AP(base_ap.tensor, base_ap.offset + p0 * pitch + off,
                   [[pstep * pitch, np_]] + [list(d) for d in dims])


def build_unet():
    nc = bacc.Bacc("TRN2", target_bir_lowering=False, debug=False)

    # --- DRAM tensors -------------------------------------------------------
    x_in = nc.dram_tensor("x", [1, N * N], STREAM_DT, kind="ExternalInput").ap()
    out_t = nc.dram_tensor("out", [N, N], F32, kind="ExternalOutput").ap()

    chans = {"enc1p": 16, "down1p": 32, "enc2p": 32, "down2p": 64,
             "enc3p": 64, "bnp": 128, "up2p": 64, "dec2p": 64,
             "up1p": 32, "dec1p": 32, "up0p": 16}
    widths = {"enc1p": 512, "down1p": 256, "enc2p": 256, "down2p": 128,
              "enc3p": 128, "bnp": 64, "up2p": 128, "dec2p": 128,
              "up1p": 256, "dec1p": 256, "up0p": 512}
    bufs = {}
    for nm in chans:
        bufs[nm] = nc.dram_tensor(nm, [chans[nm], widths[nm] ** 2], STREAM_DT,
                                  kind="Internal").ap()
    bufs["x"] = x_in

    win = {}
    for (name, kind, cin, cout, w_in, *_r) in LAYERS:
        for key, m in mm_keys(name, kind, cin, w_in):
            k = len(m["segs"]) * cin
            win[key] = nc.dram_tensor(key, [k, cout], STREAM_DT,
                                      kind="ExternalInput").ap()
        win[f"B_{name}"] = nc.dram_tensor(f"B_{name}", [128, 1], F32,
                                          kind="ExternalInput").ap()
    win["W_out4"] = nc.dram_tensor("W_out4", [128, 1], STREAM_DT,
                                   kind="ExternalInput").ap()
    win["B_out"] = nc.dram_tensor("B_out", [128, 1], F32,
                                  kind="ExternalInput").ap()

    mask_np = np.tril(np.ones((N, N), np.float32))
    mask_t = nc.inline_tensor(mask_np, name="trimask").ap()

    with TileContext(nc) as tc:
        with (
            tc.tile_pool(name="wpool", bufs=1) as wpool,
            tc.tile_pool(name="inpool", bufs=3) as inpool,
            tc.tile_pool(name="actpool", bufs=4) as actpool,
            tc.tile_pool(name="skpool", bufs=3) as skpool,
            tc.tile_pool(name="psum", bufs=6, space="PSUM") as pspool,
        ):
            # --- persistent tiles: weights, biases -------------------------
            wt = {}
            for (name, kind, cin, cout, w_in, *_r) in LAYERS:
                for key, m in mm_keys(name, kind, cin, w_in):
                    k = len(m["segs"]) * cin
                    t = wpool.tile([128, cout], STREAM_DT, tag=key, name=key)
                    nc.sync.dma_start(out=t[m["p0"]:m["p0"] + k, :],
                                      in_=win[key])
                    wt[key] = t
                t = wpool.tile([128, 1], F32, tag=f"B_{name}", name=f"B_{name}")
                nc.sync.dma_start(out=t[:, :], in_=win[f"B_{name}"])
                wt[f"B_{name}"] = t
            t = wpool.tile([128, 1], STREAM_DT, tag="W_out4", name="W_out4")
            nc.sync.dma_start(out=t[:, :], in_=win["W_out4"])
            wt["W_out4"] = t
            t = wpool.tile([128, 1], F32, tag="B_out", name="B_out")
            nc.sync.dma_start(out=t[:, :], in_=win["B_out"])
            wt["B_out"] = t

            # --- staging: padded window in SBUF, loaded once + shifted -----
            # tile row k <-> interior input row (y0-1+k); col x in [1..w] <->
            # interior col x-1; cols {0, W-1} are the zero halo ring.
            def stage(inb, cin, w, y0, Rg, blocks, tag):
                W = wp(w)
                span = (Rg + 4) * W
                nb = len(blocks)
                tin = inpool.tile([nb * cin, span], STREAM_DT, tag=tag,
                                  name="tin")
                tin_ap = tin[:, :]
                # zero ring columns of block 0 (copies propagate them)
                nc.vector.memset(
                    sub_ap(tin_ap, 0, cin, 0, [[W, Rg + 4], [W - 1, 2]]), 0.0)
                r_lo = max(0, y0 - 1)
                r_hi = min(w - 1, y0 + Rg)
                nrows = r_hi - r_lo + 1
                nc.sync.dma_start(
                    out=sub_ap(tin_ap, 0, cin, (r_lo - y0 + 1) * W + 1,
                               [[W, nrows], [1, w]]),
                    in_=sub_ap(bufs[inb], 0, cin, r_lo * w,
                               [[w, nrows], [1, w]]))
                if y0 == 0:
                    nc.vector.memset(sub_ap(tin_ap, 0, cin, 1, [[1, w]]), 0.0)
                if y0 + Rg >= w:
                    cnt = y0 + Rg - w + 1
                    nc.vector.memset(
                        sub_ap(tin_ap, 0, cin, (w - y0 + 1) * W + 1,
                               [[W, cnt], [1, w]]), 0.0)
                for ji, j in enumerate(blocks[1:], start=1):
                    nc.gpsimd.dma_start(
                        out=sub_ap(tin_ap, ji * cin, cin, 0, [[1, span - j]]),
                        in_=sub_ap(tin_ap, 0, cin, j, [[1, span - j]]))
                return tin_ap

            # --- layer emitters -------------------------------------------
            store_rr = [0]

            def store_engine():
                store_rr[0] += 1
                return nc.gpsimd if store_rr[0] % 2 else nc.sync

            def emit_s1(name, cin, cout, w, inb, outb):
                W = wp(w)
                blocks, mms = s1_plan(cin, w, name in NOBLK)
                stride = pack_stride(cout)
                pack = 128 // stride
                rpc = S1_RPC[w]
                rows_pt = 2 * pack * rpc  # 2-bank psum: 2 chunks per group
                bias = wt[f"B_{name}"][:, :]
                fuse_out = outb is None
                Rg = GROUP[name]
                for y0 in range(0, w, Rg):
                    tin_ap = stage(inb, cin, w, y0, Rg, blocks, "inb")
                    for yt0 in range(y0, y0 + Rg, rows_pt):
                        ps = pspool.tile([128, 1024], F32, tag="ps", bufs=2,
                                         name="ps")
                        for pk in range(pack):
                            col = pk * stride
                            for h in range(2):
                                y = yt0 + pk * 2 * rpc + h * rpc
                                for mi, m in enumerate(mms):
                                    K = len(m["segs"]) * cin
                                    rhs = sub_ap(tin_ap, m["p0"], K,
                                                 (y - y0) * W + m["r"],
                                                 [[W, rpc], [1, w]])
                                    nc.tensor.matmul(
                                        ps[col:col + cout,
                                           h * 512:h * 512 + rpc * w],
                                        lhsT=wt[f"W_{name}_{mi}"][m["p0"]:m["p0"] + K, 0:cout],
                                        rhs=rhs,
                                        start=(mi == 0), stop=(mi == len(mms) - 1),
                                        tile_position=(m["p0"], col))
                        act = actpool.tile([128, 1024], STREAM_DT, tag="act",
                                           name="act")
                        act_ap = act[:, :]
                        nc.scalar.activation(act_ap, ps[:, :], AF.Prelu,
                                             bias=bias, alpha=ALPHA)
                        if fuse_out:
                            emit_out_rows(act_ap, yt0)
                        else:
                            for pk in range(pack):
                                y = yt0 + pk * 2 * rpc
                                col = pk * stride
                                store_engine().dma_start(
                                    out=sub_ap(bufs[outb], 0, cout, y * w,
                                               [[1, 2 * rpc * w]]),
                                    in_=sub_ap(act_ap, col, cout, 0,
                                               [[1, 2 * rpc * w]]))

            def emit_out_rows(act_ap, y0):
                # fused 1x1 conv + bias + triangular mask for dec0 rows
                # y0..y0+7: act partition group pk holds rows y0+2pk (cols
                # 0-511) and y0+2pk+1 (cols 512-1023)
                ps2 = pspool.tile([128, 1024], F32, tag="ps2", bufs=1,
                                  name="ps2")
                for pk in range(4):
                    for h in range(2):
                        nc.tensor.matmul(
                            ps2[pk * 32:pk * 32 + 1, h * 512:(h + 1) * 512],
                            lhsT=wt["W_out4"][pk * 32:pk * 32 + 16, 0:1],
                            rhs=sub_ap(act_ap, pk * 32, 16, h * 512,
                                       [[1, 512]]),
                            start=True, stop=True,
                            tile_position=(pk * 32, pk * 32))
                actf = actpool.tile([128, 1024], F32, tag="actf", bufs=2,
                                    name="actf")
                nc.scalar.activation(actf[:, :], ps2[:, :], AF.Identity,
                                     bias=wt["B_out"][:, :])
                mt = skpool.tile([128, 1024], F32, tag="mask", name="mt")
                nc.sync.dma_start(
                    out=strided_part_ap(mt[:, :], 0, 32, 4, 0, [[1, 1024]]),
                    in_=bass.AP(mask_t.tensor, mask_t.offset + y0 * 512,
                                [[1024, 4], [1, 1024]]))
                nc.vector.tensor_mul(out=actf[:, :], in0=actf[:, :],
                                     in1=mt[:, :])
                nc.sync.dma_start(
                    out=bass.AP(out_t.tensor, out_t.offset + y0 * 512,
                                [[1024, 4], [1, 1024]]),
                    in_=strided_part_ap(actf[:, :], 0, 32, 4, 0, [[1, 1024]]))

            def emit_s2(name, cin, cout, w_in, w_out, inb, outb):
                Wi = wp(w_in)
                blocks, mms = s2_plan(cin, w_in, name in NOBLK)
                stride = pack_stride(cout)
                pack = 128 // stride
                R = 512 // w_out          # out rows per psum col-chunk
                rows_pt = 2 * pack * R    # 2-bank psum
                bias = wt[f"B_{name}"][:, :]
                Rg = GROUP[name]          # out rows per load group
                for y0 in range(0, w_out, Rg):
                    tin_ap = stage(inb, cin, w_in, 2 * y0, 2 * Rg, blocks,
                                   "inb")
                    for yt0 in range(y0, y0 + Rg, rows_pt):
                        ps = pspool.tile([128, 1024], F32, tag="ps", bufs=2,
                                         name="ps")
                        for pk in range(pack):
                            col = pk * stride
                            for h in range(2):
                                base = 2 * (yt0 + pk * 2 * R + h * R - y0) * Wi
                                for mi, m in enumerate(mms):
                                    K = len(m["segs"]) * cin
                                    rhs = sub_ap(tin_ap, m["p0"], K,
                                                 base + m["r"],
                                                 [[2 * Wi, R], [2, w_out]])
                                    nc.tensor.matmul(
                                        ps[col:col + cout,
                                           h * 512:(h + 1) * 512],
                                        lhsT=wt[f"W_{name}_{mi}"][m["p0"]:m["p0"] + K, 0:cout],
                                        rhs=rhs,
                                        start=(mi == 0), stop=(mi == len(mms) - 1),
                                        tile_position=(m["p0"], col))
                        act = actpool.tile([128, 1024], STREAM_DT, tag="act",
                                           name="act")
                        act_ap = act[:, :]
                        nc.scalar.activation(act_ap, ps[:, :], AF.Prelu,
                                             bias=bias, alpha=ALPHA)
                        for pk in range(pack):
                            col = pk * stride
                            yo = yt0 + pk * 2 * R
                            store_engine().dma_start(
                                out=sub_ap(bufs[outb], 0, cout, yo * w_out,
                                           [[1, 2 * R * w_out]]),
                                in_=sub_ap(act_ap, col, cout, 0,
                                           [[1, 2 * R * w_out]]))

            def emit_tconv(name, cin, cout, w_in, w_out, inb, outb, skipb):
                Wi = wp(w_in)
                blocks, classes = tconv_plan(cin, w_in, name in NOBLK)
                stride = pack_stride(cout)
                pack = 128 // stride
                Ri = 512 // w_in          # input rows per psum col-chunk
                rows_pt = pack * Ri
                bias = wt[f"B_{name}"][:, :]
                Rg = GROUP[name]          # input rows per load group
                for i0 in range(0, w_in, Rg):
                    tin_ap = stage(inb, cin, w_in, i0, Rg, blocks, "inb")
                    for it in range(0, Rg, rows_pt):
                        # all 4 parity classes -> one row-interleaved wide
                        # tile covering output rows [2*(i0+it), +2*rows_pt)
                        wide = actpool.tile([128, 2048], STREAM_DT,
                                            tag="wide", bufs=3, name="wide")
                        wide_ap = wide[:, :]
                        wpitch = wide_ap.ap[0][0]
                        for py, px, mms in classes:
                            ps = pspool.tile([128, 512], F32, tag="pst",
                                             bufs=2, name="ps")
                            for pk in range(pack):
                                col = pk * stride
                                base = (it + pk * Ri + 1) * Wi + 1
                                for mi, m in enumerate(mms):
                                    K = len(m["segs"]) * cin
                                    rhs = sub_ap(tin_ap, m["p0"], K,
                                                 base + m["r"],
                                                 [[Wi, Ri], [1, w_in]])
                                    nc.tensor.matmul(
                                        ps[col:col + cout, 0:Ri * w_in],
                                        lhsT=wt[f"W_{name}_c{py}{px}_{mi}"][m["p0"]:m["p0"] + K, 0:cout],
                                        rhs=rhs,
                                        start=(mi == 0), stop=(mi == len(mms) - 1),
                                        tile_position=(m["p0"], col))
                            ps_ap = ps[:, :]
                            ppitch = ps_ap.ap[0][0]
                            oap = bass.AP(wide_ap.tensor,
                                          wide_ap.offset + py * w_out + px,
                                          [[wpitch, 128], [2 * w_out, Ri],
                                           [2, w_in]])
                            iap = bass.AP(ps_ap.tensor, ps_ap.offset,
                                          [[ppitch, 128], [w_in, Ri],
                                           [1, w_in]])
                            nc.scalar.activation(oap, iap, AF.Prelu,
                                                 bias=bias, alpha=ALPHA)
                        skt = skpool.tile([128, 2048], STREAM_DT,
                                          tag="skt", name="skt")
                        skt_ap = skt[:, :]
                        for pk in range(pack):
                            io = i0 + it + pk * Ri
                            nc.gpsimd.dma_start(
                                out=sub_ap(skt_ap, pk * stride, cout, 0,
                                           [[1, 2 * Ri * w_out]]),
                                in_=sub_ap(bufs[skipb], 0, cout,
                                           2 * io * w_out,
                                           [[1, 2 * Ri * w_out]]))
                        nc.vector.tensor_add(out=wide_ap, in0=wide_ap,
                                             in1=skt_ap)
                        for pk in range(pack):
                            io = i0 + it + pk * Ri
                            nc.scalar.dma_start(
                                out=sub_ap(bufs[outb], 0, cout,
                                           2 * io * w_out,
                                           [[1, 2 * Ri * w_out]]),
                                in_=sub_ap(wide_ap, pk * stride, cout, 0,
                                           [[1, 2 * Ri * w_out]]))

            nlayers = int(os.environ.get("UNET_NLAYERS", "99"))
            for (name, kind, cin, cout, w_in, w_out, wsrc, inb, outb, skipb) in LAYERS[:nlayers]:
                if kind == "s1":
                    emit_s1(name, cin, cout, w_in, inb, outb)
                elif kind == "s2":
                    emit_s2(name, cin, cout, w_in, w_out, inb, outb)
                else:
                    emit_tconv(name, cin, cout, w_in, w_out, inb, outb, skipb)

            if nlayers < len(LAYERS):
                nc.sync.dma_start(out=out_t[:, :], in_=mask_t[:, :])

            # --- diagonal softplus patch ----------------------------------
            # softplus(x) = relu(x) + ln(1 + exp(-|x|))
            if nlayers >= len(LAYERS):
                out_flat = out_t.flatten()
                diag_ap = bass.AP(out_flat.tensor, out_flat.offset, [[513, 512]])
                dt_ = actpool.tile([1, 512], F32, tag="diag", bufs=1,
                                   name="dt_")
                nc.sync.dma_start(out=dt_[:, :], in_=diag_ap)
                ta = actpool.tile([1, 512], F32, tag="diag_a", bufs=1,
                                  name="ta")
                nc.scalar.activation(ta[:, :], dt_[:, :], AF.Abs)
                nc.scalar.activation(ta[:, :], ta[:, :], AF.Exp, scale=-1.0)
                nc.vector.tensor_scalar_add(out=ta[:, :], in0=ta[:, :], scalar1=1.0)
                nc.scalar.activation(ta[:, :], ta[:, :], AF.Ln)
                tr = actpool.tile([1, 512], F32, tag="diag_r", bufs=1,
                                  name="tr")
                nc.scalar.activation(tr[:, :], dt_[:, :], AF.Relu)
                nc.vector.tensor_add(out=tr[:, :], in0=tr[:, :], in1=ta[:, :])
                nc.sync.dma_start(out=diag_ap, in_=tr[:, :])

    nc.compile()
    return nc


_NC_CACHE = None


def get_nc():
    global _NC_CACHE
    if _NC_CACHE is None:
        _NC_CACHE = build_unet()
    return _NC_CACHE


def make_in_maps(inputs):
    wmap = prep_weights(inputs)
    x = np.asarray(inputs["x"])  # [8, 512, 512, 1] f32
    in_maps = []
    for i in range(B):
        m = dict(wmap)
        m["x"] = np.ascontiguousarray(
            x[i, :, :, 0].reshape(1, -1).astype(STREAM_NP))
        in_maps.append(m)
    return in_maps


def kernel(_trace=False, _tmpdir=None, **inputs):
    nc = get_nc()
    in_maps = make_in_maps(inputs)
    res = run_bass_kernel_spmd(nc, in_maps, core_ids=list(range(B)),
                               trace=_trace, tmpdir=_tmpdir)
    out = np.stack([res.results[i]["out"] for i in range(B)], axis=0)
    out = out[:, :, :, None].astype(np.float32)
    if _trace:
        return out, res
    return out


# revision 26
# speedup vs baseline: 1.2760x; 1.2760x over previous
"""PreconditionerSparseUNet on 8 TRN2 NeuronCores.

Sharding: data-parallel over batch (8 images, 1 per core). Each core runs the
full U-Net on its own 512x512x1 image; weights are replicated.

Per-core implementation notes (v3 — HBM-traffic-minimized):
- Feature maps live in DRAM UNPADDED as [C, w*w] fp16. No halo ring is ever
  stored: loads materialize a padded (w+2)-pitch window in SBUF, where ring
  columns / halo rows are zeroed by tiny strided memsets per group.
- Each conv group loads its input rows from HBM ONCE; the shifted partition
  blocks needed for K-folding are produced by on-chip SBUF->SBUF copies
  (kills the 2-3x HBM read amplification of the naive scheme).
- 3x3 convs are a few matmuls per output chunk: input channel blocks
  replicated into SBUF partitions at shifted offsets so one matmul contracts
  several taps. rhs/lhsT fp16 (full PE rate), PSUM accumulates fp32.
- Conv outputs of M<=64 channels are packed 4x/2x along PSUM partitions so
  the bias+LeakyReLU epilogue runs once per 128-partition tile. Output rows
  are row-aligned: stores are contiguous ~1KB runs into the unpadded layout.
- Transposed convs are decomposed into 4 output-parity classes written with
  stride-2 row access; skip-add happens after LeakyReLU, before store.
- The final 1x1 conv + triangular masking is FUSED into dec0's epilogue
  (dec0's feature map never round-trips DRAM); the diagonal gets a softplus
  patch at the end.
"""

import os

import numpy as np

import concourse.bass as bass
import concourse.bacc as bacc
import concourse.mybir as mybir
from concourse.tile import TileContext
from concourse.bass_utils import run_bass_kernel_spmd

AF = mybir.ActivationFunctionType
F32 = mybir.dt.float32
F16 = mybir.dt.float16

STREAM_DT = F16
STREAM_NP = np.float16

N = 512
B = 8
ALPHA = 0.01

CH = [1, 16, 32, 64, 128, 1]


def wp(w):
    return w + 2


# ----------------------------------------------------------------------------
# Matmul plans. Each matmul: dict(p0, segs, r) where segs is a list of
# (ky, kx) weight-slice picks (or None for zero rows), one per Cin-sized
# block slot starting at partition p0; r is the rhs offset in padded-window
# elements relative to the chunk base.
# ----------------------------------------------------------------------------

# layers whose staging skips shifted-block replication: conv taps become 9
# accumulating K=cin matmuls with rhs offsets (cheap PE) instead of 2-3x
# SBUF->SBUF copy traffic (expensive DMA under activity throttling)
NOBLK = set()  # blockless staging costs more PE than it saves in DMA


def s1_plan(cin, w, noblk=False):
    W = wp(w)
    if noblk:
        blocks = [0]
        mms = [dict(p0=0, segs=[(ky, kx)], r=ky * W + kx)
               for ky in range(3) for kx in range(3)]
        return blocks, mms
    if cin == 1:
        blocks = [ky * W + kx for ky in range(3) for kx in range(3)]
        mms = [dict(p0=0, segs=[(ky, kx) for ky in range(3) for kx in range(3)], r=0)]
    elif cin <= 32:
        blocks = [0, 1, 2]
        mms = [dict(p0=0, segs=[(ky, 0), (ky, 1), (ky, 2)], r=ky * W)
               for ky in range(3)]
    elif cin == 64:
        blocks = [0, 1]
        mms = []
        for ky in range(3):
            mms.append(dict(p0=0, segs=[(ky, 0), (ky, 1)], r=ky * W))
            mms.append(dict(p0=0, segs=[(ky, 2)], r=ky * W + 2))
    else:
        raise ValueError(cin)
    return blocks, mms


def s2_plan(cin, w_in, noblk=False):
    # identical tap algebra to s1 (offsets are in padded input-window space)
    return s1_plan(cin, w_in, noblk)


def pmap(parity, d):
    # transposed conv: output parity p, input-tap delta d -> kernel index
    if parity == 0:
        return 1 if d == 0 else None
    return 0 if d == 0 else 2


def tconv_plan(cin, w_in, noblk=False):
    W = wp(w_in)
    if noblk or cin == 128:
        blocks = [0]
    elif cin == 64:
        blocks = [0, 1]  # dj shifts
    elif cin == 32:
        blocks = [0, 1, W, W + 1]  # (di,dj) = (0,0),(0,1),(1,0),(1,1)
    else:
        raise ValueError(cin)
    classes = []
    for py in range(2):
        for px in range(2):
            dis = [d for d in range(2) if pmap(py, d) is not None]
            djs = [d for d in range(2) if pmap(px, d) is not None]
            mms = []
            if noblk or cin == 128:
                for di in dis:
                    for dj in djs:
                        mms.append(dict(p0=0, segs=[(pmap(py, di), pmap(px, dj))],
                                        r=di * W + dj))
            elif cin == 64:
                for di in dis:
                    if len(djs) == 2:
                        mms.append(dict(p0=0,
                                        segs=[(pmap(py, di), pmap(px, 0)),
                                              (pmap(py, di), pmap(px, 1))],
                                        r=di * W))
                    else:
                        mms.append(dict(p0=0, segs=[(pmap(py, di), 1)],
                                        r=di * W))
            else:  # cin == 32, all four blocks resident
                if py == 0 and px == 0:
                    mms = [dict(p0=0, segs=[(1, 1)], r=0)]
                elif py == 0 and px == 1:
                    mms = [dict(p0=0, segs=[(1, 0), (1, 2)], r=0)]
                elif py == 1 and px == 0:
                    # one K=128 matmul over all four blocks; blocks holding
                    # dj=1 shifts get zero weight rows
                    mms = [dict(p0=0, segs=[(0, 1), None, (2, 1), None], r=0)]
                else:
                    mms = [dict(p0=0, segs=[(0, 0), (0, 2), (2, 0), (2, 2)], r=0)]
            classes.append((py, px, mms))
    return blocks, classes


# Layer table: (name, kind, cin, cout, w_in, w_out, wsrc, in, out, skip)
LAYERS = [
    ("enc1", "s1", 1, 16, 512, 512, "w_enc1", "x", "enc1p", None),
    ("down1", "s2", 16, 32, 512, 256, "w_down1", "enc1p", "down1p", None),
    ("enc2", "s1", 32, 32, 256, 256, "w_enc2", "down1p", "enc2p", None),
    ("down2", "s2", 32, 64, 256, 128, "w_down2", "enc2p", "down2p", None),
    ("enc3", "s1", 64, 64, 128, 128, "w_enc3", "down2p", "enc3p", None),
    ("bn", "s2", 64, 128, 128, 64, "w_bn", "enc3p", "bnp", None),
    ("up2", "tc", 128, 64, 64, 128, "w_up2", "bnp", "up2p", "enc3p"),
    ("dec2", "s1", 64, 64, 128, 128, "w_dec2", "up2p", "dec2p", None),
    ("up1", "tc", 64, 32, 128, 256, "w_up1", "dec2p", "up1p", "enc2p"),
    ("dec1", "s1", 32, 32, 256, 256, "w_dec1", "up1p", "dec1p", None),
    ("up0", "tc", 32, 16, 256, 512, "w_up0", "dec1p", "up0p", "enc1p"),
    ("dec0", "s1", 16, 16, 512, 512, "w_dec0", "up0p", None, None),
]

# group sizes: s1 = output rows; s2 = OUTPUT rows; tc = input rows
GROUP = {
    "enc1": 32, "down1": 16, "enc2": 32, "down2": 16, "enc3": 64,
    "bn": 16, "up2": 64, "dec2": 64, "up1": 32, "dec1": 32,
    "up0": 16, "dec0": 32,
}

# s1 rows per psum column-chunk (rpc * w <= 512)
S1_RPC = {512: 1, 256: 2, 128: 4}


def pack_stride(cout):
    return 32 if cout <= 32 else (64 if cout == 64 else 128)


def layer_plan(kind, cin, w_in, name=None):
    noblk = name in NOBLK
    if kind == "s1":
        return s1_plan(cin, w_in, noblk)
    if kind == "s2":
        return s2_plan(cin, w_in, noblk)
    return tconv_plan(cin, w_in, noblk)


def mm_keys(name, kind, cin, w_in):
    """Enumerate (key, m) for every matmul of a layer."""
    out = []
    if kind in ("s1", "s2"):
        _, mms = layer_plan(kind, cin, w_in, name)
        for i, m in enumerate(mms):
            out.append((f"W_{name}_{i}", m))
    else:
        _, classes = layer_plan(kind, cin, w_in, name)
        for py, px, mms in classes:
            for i, m in enumerate(mms):
                out.append((f"W_{name}_c{py}{px}_{i}", m))
    return out


# ----------------------------------------------------------------------------
# Host-side input prep
# ----------------------------------------------------------------------------

def prep_weights(inputs):
    """Build per-matmul lhsT arrays (fp16) and packed bias vectors."""
    wmap = {}
    for (name, kind, cin, cout, w_in, w_out, wsrc, *_rest) in LAYERS:
        w = np.asarray(inputs[wsrc])  # [3,3,cin,cout]
        for key, m in mm_keys(name, kind, cin, w_in):
            segs = []
            for s in m["segs"]:
                if s is None:
                    segs.append(np.zeros((cin, cout), np.float32))
                else:
                    segs.append(w[s[0], s[1]])
            wmap[key] = np.ascontiguousarray(
                np.concatenate(segs, axis=0).astype(STREAM_NP))
        bsrc = "b_" + wsrc[2:]
        b = np.asarray(inputs[bsrc]).astype(np.float32)
        stride = pack_stride(cout)
        b128 = np.zeros((128, 1), np.float32)
        for pk in range(128 // stride):
            b128[pk * stride: pk * stride + cout, 0] = b
        wmap[f"B_{name}"] = b128
    # final 1x1 conv: replicate the [16,1] weight at 4 partition bases so the
    # fused out-matmuls can run on diagonal PE tiles against packed act rows
    w_out4 = np.zeros((128, 1), np.float32)
    for pk in range(4):
        w_out4[pk * 32: pk * 32 + 16, 0] = np.asarray(inputs["w_out"]).reshape(16)
    wmap["W_out4"] = w_out4.astype(STREAM_NP)
    wmap["B_out"] = np.full((128, 1), float(np.asarray(inputs["b_out"])[0]),
                            np.float32)
    return wmap


# ----------------------------------------------------------------------------
# Kernel builder
# ----------------------------------------------------------------------------

def sub_ap(base_ap, p0, np_, off, dims):
    """AP over partitions [p0,p0+np_) of base_ap with free dims at elem off."""
    pitch = base_ap.ap[0][0]
    return bass.AP(base_ap.tensor, base_ap.offset + p0 * pitch + off,
                   [[pitch, np_]] + [list(d) for d in dims])


def strided_part_ap(base_ap, p0, pstep, np_, off, dims):
    """AP over partitions p0, p0+pstep, ... of base_ap."""
    pitch = base_ap.ap[0][0]
    return bass.AP(base_ap.tensor, base_ap.offset + p0 * pitch + off,
                   [[pstep * pitch, np_]] + [list(d) for d in dims])


def build_unet():
    nc = bacc.Bacc("TRN2", target_bir_lowering=False, debug=False)

    # --- DRAM tensors -------------------------------------------------------
    x_in = nc.dram_tensor("x", [1, N * N], STREAM_DT, kind="ExternalInput").ap()
    out_t = nc.dram_tensor("out", [N, N], F32, kind="ExternalOutput").ap()

    chans = {"enc1p": 16, "down1p": 32, "enc2p": 32, "down2p": 64,
             "enc3p": 64, "bnp": 128, "up2p": 64, "dec2p": 64,
             "up1p": 32, "dec1p": 32, "up0p": 16}
    widths = {"enc1p": 512, "down1p": 256, "enc2p": 256, "down2p": 128,
              "enc3p": 128, "bnp": 64, "up2p": 128, "dec2p": 128,
              "up1p": 256, "dec1p": 256, "up0p": 512}
    bufs = {}
    for nm in chans:
        bufs[nm] = nc.dram_tensor(nm, [chans[nm], widths[nm] ** 2], STREAM_DT,
                                  kind="Internal").ap()
    bufs["x"] = x_in

    win = {}
    for (name, kind, cin, cout, w_in, *_r) in LAYERS:
        for key, m in mm_keys(name, kind, cin, w_in):
            k = len(m["segs"]) * cin
            win[key] = nc.dram_tensor(key, [k, cout], STREAM_DT,
                                      kind="ExternalInput").ap()
        win[f"B_{name}"] = nc.dram_tensor(f"B_{name}", [128, 1], F32,
                                          kind="ExternalInput").ap()
    win["W_out4"] = nc.dram_tensor("W_out4", [128, 1], STREAM_DT,
                                   kind="ExternalInput").ap()
    win["B_out"] = nc.dram_tensor("B_out", [128, 1], F32,
                                  kind="ExternalInput").ap()

    mask_np = np.tril(np.ones((N, N), np.float32))
    mask_t = nc.inline_tensor(mask_np, name="trimask").ap()

    with TileContext(nc) as tc:
        with (
            tc.tile_pool(name="wpool", bufs=1) as wpool,
            tc.tile_pool(name="inpool", bufs=3) as inpool,
            tc.tile_pool(name="actpool", bufs=4) as actpool,
            tc.tile_pool(name="skpool", bufs=3) as skpool,
            tc.tile_pool(name="psum", bufs=6, space="PSUM") as pspool,
        ):
            # --- persistent tiles: weights, biases -------------------------
            wt = {}
            for (name, kind, cin, cout, w_in, *_r) in LAYERS:
                for key, m in mm_keys(name, kind, cin, w_in):
                    k = len(m["segs"]) * cin
                    t = wpool.tile([128, cout], STREAM_DT, tag=key, name=key)
                    nc.sync.dma_start(out=t[m["p0"]:m["p0"] + k, :],
                                      in_=win[key])
                    wt[key] = t
                t = wpool.tile([128, 1], F32, tag=f"B_{name}", name=f"B_{name}")
                nc.sync.dma_start(out=t[:, :], in_=win[f"B_{name}"])
                wt[f"B_{name}"] = t
            t = wpool.tile([128, 1], STREAM_DT, tag="W_out4", name="W_out4")
            nc.sync.dma_start(out=t[:, :], in_=win["W_out4"])
            wt["W_out4"] = t
            t = wpool.tile([128, 1], F32, tag="B_out", name="B_out")
            nc.sync.dma_start(out=t[:, :], in_=win["B_out"])
            wt["B_out"] = t

            # --- staging: padded window in SBUF, each shifted block loaded
            # directly from HBM (engine-parallel, no SBUF port contention).
            # Block b = rs*W + cs: tile row k <-> interior row (y0-1+k+rs);
            # tile col x <-> interior col (x+cs-1). Halo/ring stay zero via
            # small strided memsets.
            load_rr = [0]

            def stage(inb, cin, w, y0, Rg, blocks, tag):
                W = wp(w)
                span = (Rg + 2) * W
                nb = len(blocks)
                tin = inpool.tile([nb * cin, span], STREAM_DT, tag=tag,
                                  name="tin")
                tin_ap = tin[:, :]
                geo = []
                for b in blocks:
                    rs, cs = divmod(b, W)
                    geo.append((rs, cs,
                                max(0, 1 - cs), min(W - 1, w - cs),
                                max(0, 1 - y0 - rs), min(Rg + 1, w - y0 - rs)))
                # blanket edge memsets over ALL blocks (engine ops must start
                # at partition 0); each block's load overwrites its data part
                np_ = nb * cin
                if any(g[2] > 0 for g in geo):
                    nc.vector.memset(
                        sub_ap(tin_ap, 0, np_, 0, [[W, Rg + 2], [1, 1]]), 0.0)
                xhi_min = min(g[3] for g in geo)
                if xhi_min < W - 1:
                    nc.vector.memset(
                        sub_ap(tin_ap, 0, np_, xhi_min + 1,
                               [[W, Rg + 2], [1, W - 1 - xhi_min]]), 0.0)
                klo_max = max(g[4] for g in geo)
                if klo_max > 0:
                    nc.vector.memset(
                        sub_ap(tin_ap, 0, np_, 0, [[1, klo_max * W]]), 0.0)
                khi_min = min(g[5] for g in geo)
                if khi_min < Rg + 1:
                    nc.vector.memset(
                        sub_ap(tin_ap, 0, np_, (khi_min + 1) * W,
                               [[1, (Rg + 1 - khi_min) * W]]), 0.0)
                for ji, (rs, cs, xlo, xhi, klo, khi) in enumerate(geo):
                    load_rr[0] += 1
                    eng = nc.sync if load_rr[0] % 2 else nc.gpsimd
                    eng.dma_start(
                        out=sub_ap(tin_ap, ji * cin, cin, klo * W + xlo,
                                   [[W, khi - klo + 1], [1, xhi - xlo + 1]]),
                        in_=sub_ap(bufs[inb], 0, cin,
                                   (y0 - 1 + klo + rs) * w + (xlo + cs - 1),
                                   [[w, khi - klo + 1], [1, xhi - xlo + 1]]))
                return tin_ap

            # --- layer emitters -------------------------------------------
            store_rr = [0]

            def store_engine():
                store_rr[0] += 1
                return nc.gpsimd if store_rr[0] % 2 else nc.sync

            def emit_s1(name, cin, cout, w, inb, outb):
                W = wp(w)
                blocks, mms = s1_plan(cin, w, name in NOBLK)
                stride = pack_stride(cout)
                pack = 128 // stride
                rpc = S1_RPC[w]
                rows_pt = 2 * pack * rpc  # 2-bank psum: 2 chunks per group
                bias = wt[f"B_{name}"][:, :]
                fuse_out = outb is None
                Rg = GROUP[name]
                for y0 in range(0, w, Rg):
                    tin_ap = stage(inb, cin, w, y0, Rg, blocks, "inb")
                    for yt0 in range(y0, y0 + Rg, rows_pt):
                        ps = pspool.tile([128, 1024], F32, tag="ps", bufs=2,
                                         name="ps")
                        for pk in range(pack):
                            col = pk * stride
                            for h in range(2):
                                y = yt0 + pk * 2 * rpc + h * rpc
                                for mi, m in enumerate(mms):
                                    K = len(m["segs"]) * cin
                                    rhs = sub_ap(tin_ap, m["p0"], K,
                                                 (y - y0) * W + m["r"],
                                                 [[W, rpc], [1, w]])
                                    nc.tensor.matmul(
                                        ps[col:col + cout,
                                           h * 512:h * 512 + rpc * w],
                                        lhsT=wt[f"W_{name}_{mi}"][m["p0"]:m["p0"] + K, 0:cout],
                                        rhs=rhs,
                                        start=(mi == 0), stop=(mi == len(mms) - 1),
                                        tile_position=(m["p0"], col))
                        act = actpool.tile([128, 1024], STREAM_DT, tag="act",
                                           name="act")
                        act_ap = act[:, :]
                        nc.scalar.activation(act_ap, ps[:, :], AF.Prelu,
                                             bias=bias, alpha=ALPHA)
                        if fuse_out:
                            emit_out_rows(act_ap, yt0)
                        else:
                            for pk in range(pack):
                                y = yt0 + pk * 2 * rpc
                                col = pk * stride
                                store_engine().dma_start(
                                    out=sub_ap(bufs[outb], 0, cout, y * w,
                                               [[1, 2 * rpc * w]]),
                                    in_=sub_ap(act_ap, col, cout, 0,
                                               [[1, 2 * rpc * w]]))

            def emit_out_rows(act_ap, y0):
                # fused 1x1 conv + bias + triangular mask for dec0 rows
                # y0..y0+7: act partition group pk holds rows y0+2pk (cols
                # 0-511) and y0+2pk+1 (cols 512-1023)
                ps2 = pspool.tile([128, 1024], F32, tag="ps2", bufs=1,
                                  name="ps2")
                for pk in range(4):
                    for h in range(2):
                        nc.tensor.matmul(
                            ps2[pk * 32:pk * 32 + 1, h * 512:(h + 1) * 512],
                            lhsT=wt["W_out4"][pk * 32:pk * 32 + 16, 0:1],
                            rhs=sub_ap(act_ap, pk * 32, 16, h * 512,
                                       [[1, 512]]),
                            start=True, stop=True,
                            tile_position=(pk * 32, pk * 32))
                actf = actpool.tile([128, 1024], F32, tag="actf", bufs=2,
                                    name="actf")
                nc.scalar.activation(actf[:, :], ps2[:, :], AF.Identity,
                                     bias=wt["B_out"][:, :])
                mt = skpool.tile([128, 1024], F32, tag="mask", name="mt")
                nc.sync.dma_start(
                    out=strided_part_ap(mt[:, :], 0, 32, 4, 0, [[1, 1024]]),
                    in_=bass.AP(mask_t.tensor, mask_t.offset + y0 * 512,
                                [[1024, 4], [1, 1024]]))
                nc.vector.tensor_mul(out=actf[:, :], in0=actf[:, :],
                                     in1=mt[:, :])
                nc.sync.dma_start(
                    out=bass.AP(out_t.tensor, out_t.offset + y0 * 512,
                                [[1024, 4], [1, 1024]]),
                    in_=strided_part_ap(actf[:, :], 0, 32, 4, 0, [[1, 1024]]))

            def emit_s2(name, cin, cout, w_in, w_out, inb, outb):
                Wi = wp(w_in)
                blocks, mms = s2_plan(cin, w_in, name in NOBLK)
                stride = pack_stride(cout)
                pack = 128 // stride
                R = 512 // w_out          # out rows per psum col-chunk
                rows_pt = 2 * pack * R    # 2-bank psum
                bias = wt[f"B_{name}"][:, :]
                Rg = GROUP[name]          # out rows per load group
                for y0 in range(0, w_out, Rg):
                    tin_ap = stage(inb, cin, w_in, 2 * y0, 2 * Rg, blocks,
                                   "inb")
                    for yt0 in range(y0, y0 + Rg, rows_pt):
                        ps = pspool.tile([128, 1024], F32, tag="ps", bufs=2,
                                         name="ps")
                        for pk in range(pack):
                            col = pk * stride
                            for h in range(2):
                                base = 2 * (yt0 + pk * 2 * R + h * R - y0) * Wi
                                for mi, m in enumerate(mms):
                                    K = len(m["segs"]) * cin
                                    rhs = sub_ap(tin_ap, m["p0"], K,
                                                 base + m["r"],
                                                 [[2 * Wi, R], [2, w_out]])
                                    nc.tensor.matmul(
                                        ps[col:col + cout,
                                           h * 512:(h + 1) * 512],
                                        lhsT=wt[f"W_{name}_{mi}"][m["p0"]:m["p0"] + K, 0:cout],
                                        rhs=rhs,
                                        start=(mi == 0), stop=(mi == len(mms) - 1),
                                        tile_position=(m["p0"], col))
                        act = actpool.tile([128, 1024], STREAM_DT, tag="act",
                                           name="act")
                        act_ap = act[:, :]
                        nc.scalar.activation(act_ap, ps[:, :], AF.Prelu,
                                             bias=bias, alpha=ALPHA)
                        for pk in range(pack):
                            col = pk * stride
                            yo = yt0 + pk * 2 * R
                            store_engine().dma_start(
                                out=sub_ap(bufs[outb], 0, cout, yo * w_out,
                                           [[1, 2 * R * w_out]]),
                                in_=sub_ap(act_ap, col, cout, 0,
                                           [[1, 2 * R * w_out]]))

            def emit_tconv(name, cin, cout, w_in, w_out, inb, outb, skipb):
                Wi = wp(w_in)
                blocks, classes = tconv_plan(cin, w_in, name in NOBLK)
                stride = pack_stride(cout)
                pack = 128 // stride
                Ri = 512 // w_in          # input rows per psum col-chunk
                rows_pt = pack * Ri
                bias = wt[f"B_{name}"][:, :]
                Rg = GROUP[name]          # input rows per load group
                for i0 in range(0, w_in, Rg):
                    tin_ap = stage(inb, cin, w_in, i0, Rg, blocks, "inb")
                    for it in range(0, Rg, rows_pt):
                        # all 4 parity classes -> one row-interleaved wide
                        # tile covering output rows [2*(i0+it), +2*rows_pt)
                        wide = actpool.tile([128, 2048], STREAM_DT,
                                            tag="wide", bufs=3, name="wide")
                        wide_ap = wide[:, :]
                        wpitch = wide_ap.ap[0][0]
                        for py, px, mms in classes:
                            ps = pspool.tile([128, 512], F32, tag="pst",
                                             bufs=2, name="ps")
                            for pk in range(pack):
                                col = pk * stride
                                base = (it + pk * Ri + 1) * Wi + 1
                                for mi, m in enumerate(mms):
                                    K = len(m["segs"]) * cin
                                    rhs = sub_ap(tin_ap, m["p0"], K,
                                                 base + m["r"],
                                                 [[Wi, Ri], [1, w_in]])
                                    nc.tensor.matmul(
                                        ps[col:col + cout, 0:Ri * w_in],
                                        lhsT=wt[f"W_{name}_c{py}{px}_{mi}"][m["p0"]:m["p0"] + K, 0:cout],
                                        rhs=rhs,
                                        start=(mi == 0), stop=(mi == len(mms) - 1),
                                        tile_position=(m["p0"], col))
                            ps_ap = ps[:, :]
                            ppitch = ps_ap.ap[0][0]
                            oap = bass.AP(wide_ap.tensor,
                                          wide_ap.offset + py * w_out + px,
                                          [[wpitch, 128], [2 * w_out, Ri],
                                           [2, w_in]])
                            iap = bass.AP(ps_ap.tensor, ps_ap.offset,
                                          [[ppitch, 128], [w_in, Ri],
                                           [1, w_in]])
                            nc.scalar.activation(oap, iap, AF.Prelu,
                                                 bias=bias, alpha=ALPHA)
                        skt = skpool.tile([128, 2048], STREAM_DT,
                                          tag="skt", name="skt")
                        skt_ap = skt[:, :]
                        for pk in range(pack):
                            io = i0 + it + pk * Ri
                            nc.gpsimd.dma_start(
                                out=sub_ap(skt_ap, pk * stride, cout, 0,
                                           [[1, 2 * Ri * w_out]]),
                                in_=sub_ap(bufs[skipb], 0, cout,
                                           2 * io * w_out,
                                           [[1, 2 * Ri * w_out]]))
                        nc.vector.tensor_add(out=wide_ap, in0=wide_ap,
                                             in1=skt_ap)
                        for pk in range(pack):
                            io = i0 + it + pk * Ri
                            nc.scalar.dma_start(
                                out=sub_ap(bufs[outb], 0, cout,
                                           2 * io * w_out,
                                           [[1, 2 * Ri * w_out]]),
                                in_=sub_ap(wide_ap, pk * stride, cout, 0,
                                           [[1, 2 * Ri * w_out]]))

            nlayers = int(os.environ.get("UNET_NLAYERS", "99"))
            for (name, kind, cin, cout, w_in, w_out, wsrc, inb, outb, skipb) in LAYERS[:nlayers]:
                if kind == "s1":
                    emit_s1(name, cin, cout, w_in, inb, outb)
                elif kind == "s2":
                    emit_s2(name, cin, cout, w_in, w_out, inb, outb)
                else:
                    emit_tconv(name, cin, cout, w_in, w_out, inb, outb, skipb)

            if nlayers < len(LAYERS):
                nc.sync.dma_start(out=out_t[:, :], in_=mask_t[:, :])

            # --- diagonal softplus patch ----------------------------------
            # softplus(x) = relu(x) + ln(1 + exp(-|x|))
            if nlayers >= len(LAYERS):
                out_flat = out_t.flatten()
                diag_ap = bass.AP(out_flat.tensor, out_flat.offset, [[513, 512]])
                dt_ = actpool.tile([1, 512], F32, tag="diag", bufs=1,
                                   name="dt_")
                nc.sync.dma_start(out=dt_[:, :], in_=diag_ap)
                ta = actpool.tile([1, 512], F32, tag="diag_a", bufs=1,
                                  name="ta")
                nc.scalar.activation(ta[:, :], dt_[:, :], AF.Abs)
                nc.scalar.activation(ta[:, :], ta[:, :], AF.Exp, scale=-1.0)
                nc.vector.tensor_scalar_add(out=ta[:, :], in0=ta[:, :], scalar1=1.0)
                nc.scalar.activation(ta[:, :], ta[:, :], AF.Ln)
                tr = actpool.tile([1, 512], F32, tag="diag_r", bufs=1,
                                  name="tr")
                nc.scalar.activation(tr[:, :], dt_[:, :], AF.Relu)
                nc.vector.tensor_add(out=tr[:, :], in0=tr[:, :], in1=ta[:, :])
                nc.sync.dma_start(out=diag_ap, in_=tr[:, :])

    nc.compile()
    return nc


_NC_CACHE = None


def get_nc():
    global _NC_CACHE
    if _NC_CACHE is None:
        _NC_CACHE = build_unet()
    return _NC_CACHE


def make_in_maps(inputs):
    wmap = prep_weights(inputs)
    x = np.asarray(inputs["x"])  # [8, 512, 512, 1] f32
    in_maps = []
    for i in range(B):
        m = dict(wmap)
        m["x"] = np.ascontiguousarray(
            x[i, :, :, 0].reshape(1, -1).astype(STREAM_NP))
        in_maps.append(m)
    return in_maps


def kernel(_trace=False, _tmpdir=None, **inputs):
    nc = get_nc()
    in_maps = make_in_maps(inputs)
    res = run_bass_kernel_spmd(nc, in_maps, core_ids=list(range(B)),
                               trace=_trace, tmpdir=_tmpdir)
    out = np.stack([res.results[i]["out"] for i in range(B)], axis=0)
    out = out[:, :, :, None].astype(np.float32)
    if _trace:
        return out, res
    return out
